# revision 4
# baseline (speedup 1.0000x reference)
"""Trainium2 Bass kernel for dynamic-depthwise + static conv module.

Computation (per batch b, channel c):
  hid  = leaky_relu(k_v @ W1.T, 0.1)
  kern = (hid @ W2.T).reshape(b*c, 3, 3)        # per-(b,c) dynamic 3x3
  dyn  = leaky_relu(depthwise3x3(x, kern), 0.1)
  res  = conv3x3(x, conv_w) + conv_b
  out  = dyn + res

Sharding: pure data-parallel, B=16 over 8 cores (2 batches/core).

Per-core device layout:
  x_pad  [128 part = 2 batches x 64 ch, 194*194 zero-padded f32r]
  For each spatial tile (2 padded rows, N=388) and each of 9 taps:
    one f32r matmul per batch, lhsT[k=ci, m] = [conv_w tap | diag(kern)]
    -> PSUM bank per batch: parts 0-63 = static conv, parts 64-127 = dynamic.
  Two batches run concurrently via PE row tiling (rows 0-63 / 64-127).
  ACT evicts dynamic half with Prelu(0.1); an identity matmul accumulates
  it onto the static half (cross-partition add on the PE); DVE adds bias
  and compacts padding on final eviction; DMA to HBM.
"""
import numpy as np

import concourse.bass as bass
import concourse.tile as tile
import concourse.mybir as mybir

F32 = mybir.dt.float32
F32R = mybir.dt.float32r

B, C, H, W = 16, 64, 192, 192
NCORES = 8
BLOC = B // NCORES          # batches per core
WP = W + 2                  # padded row width
HP = H + 2
PADQ = WP * HP
G = 4                       # guard elems each side of padded buffer
HW = H * W
NTILE = 388                 # 2 padded rows per tile
NT = H // 2                 # 96 tiles

TAPS = [(dy, dx) for dy in (-1, 0, 1) for dx in (-1, 0, 1)]


def _legalize_waits(nc, max_waits=1, evsem_waits=2):
    """This walrus build rejects >1 sync wait on most instructions (2 on
    EventSemaphore). Spill excess waits onto same-engine EventSemaphores
    placed immediately before the instruction."""
    for f in nc.m.functions:
        for bb in f.blocks:
            new_insts = []
            for inst in bb.instructions:
                si = inst.sync_info
                if si is not None and si.on_wait and len(si.on_wait) > max_waits:
                    waits = list(si.on_wait)
                    keep = waits[-max_waits:]
                    spill = waits[:-max_waits]
                    while spill:
                        chunk, spill = spill[:evsem_waits], spill[evsem_waits:]
                        ev = mybir.InstEventSemaphore(
                            name=nc.get_next_instruction_name(),
                            engine=inst.engine,
                            ins=[],
                            outs=[],
                            sync_info=mybir.SyncInfo(on_wait=chunk, on_update=[]),
                        )
                        nc.register_instruction(ev)
                        new_insts.append(ev)
                    inst.sync_info = mybir.SyncInfo(
                        on_wait=keep, on_update=list(si.on_update or [])
                    )
                new_insts.append(inst)
            bb.instructions[:] = new_insts


def _build_nc():
    nc = bass.Bass()
    xs = nc.dram_tensor("xs", (128, HW), F32, kind="ExternalInput")
    kvT = nc.dram_tensor("kvT", (64, BLOC), F32, kind="ExternalInput")
    w1t = nc.dram_tensor("w1t", (64, 64), F32, kind="ExternalInput")
    w2t = nc.dram_tensor("w2t", (64, 576), F32, kind="ExternalInput")
    wstat = nc.dram_tensor("wstat", (128, 9 * 128), F32, kind="ExternalInput")
    ident = nc.dram_tensor("ident", (128, 64), F32, kind="ExternalInput")
    biasd = nc.dram_tensor("biasd", (128, 1), F32, kind="ExternalInput")
    out = nc.dram_tensor("out", (128, HW), F32, kind="ExternalOutput")

    with tile.TileContext(nc) as tc:
        with (
            tc.tile_pool(name="big", bufs=1) as big,
            tc.tile_pool(name="wpool", bufs=1) as wpool,
            tc.tile_pool(name="work", bufs=3) as work,
            tc.tile_pool(name="pmain", bufs=3, space="PSUM") as pmain,
            tc.tile_pool(name="pmlp", bufs=2, space="PSUM") as pmlp,
        ):
            # ---- persistent tiles ----
            x_pad = big.tile([128, G + PADQ + G], F32R, tag="x_pad")
            wbuf = wpool.tile([128, 9 * 128], F32R, tag="wbuf")
            id_t = wpool.tile([128, 64], F32R, tag="id_t")
            bias_t = wpool.tile([128, 1], F32, tag="bias_t")
            kvT_t = wpool.tile([64, BLOC], F32R, tag="kvT_t")
            w1t_t = wpool.tile([64, 64], F32R, tag="w1t_t")
            w2t_t = wpool.tile([64, 576], F32R, tag="w2t_t")
            kern_flat = wpool.tile([BLOC, 576], F32, tag="kern_flat")
            kern128 = wpool.tile([128, 9], F32, tag="kern128")
            hidT = wpool.tile([64, BLOC], F32R, tag="hidT")

            # ---- constant / weight loads ----
            nc.gpsimd.dma_start(wbuf[:], wstat[:])
            nc.gpsimd.dma_start(id_t[:], ident[:])
            nc.gpsimd.dma_start(bias_t[:], biasd[:])
            nc.gpsimd.dma_start(kvT_t[:], kvT[:])
            nc.gpsimd.dma_start(w1t_t[:], w1t[:])
            nc.gpsimd.dma_start(w2t_t[:], w2t[:])

            # ---- zero padding regions of x_pad ----
            # left guard + top pad row
            nc.gpsimd.memset(x_pad[:, 0:G + WP].bitcast(mybir.dt.uint32), 0)
            # bottom pad row + right guard
            nc.gpsimd.memset(
                x_pad[:, G + (HP - 1) * WP:G + PADQ + G].bitcast(mybir.dt.uint32), 0)
            # vertical pad columns: pairs (wp=193,row r),(wp=0,row r+1)
            vcols = x_pad[:, G + WP - 1:G + WP - 1 + (HP - 1) * WP]
            vcols = vcols.rearrange("p (r w) -> p r w", w=WP)[:, :, 0:2]
            nc.gpsimd.memset(vcols.bitcast(mybir.dt.uint32), 0)

            # ---- x interior load (strided, f32 -> f32r cast), split for
            # multi-queue parallelism ----
            xdst = x_pad[:, G + WP:G + WP + H * WP]
            xdst = xdst.rearrange("p (r w) -> p r w", w=WP)[:, :, 1:1 + W]
            xsrc = xs[:, :].rearrange("p (r w) -> p r w", w=W)
            NSPLIT = 16
            rows_per = H // NSPLIT
            for s in range(NSPLIT):
                r0, r1 = s * rows_per, (s + 1) * rows_per
                nc.gpsimd.dma_start(xdst[:, r0:r1, :], xsrc[:, r0:r1, :])

            # ---- MLP: kern = (lrelu(k_v @ W1.T) @ W2.T) ----
            p_hid = pmlp.tile([64, 512], F32, tag="pmlp")
            nc.tensor.matmul(p_hid[0:64, 0:BLOC], w1t_t[:], kvT_t[:],
                             start=True, stop=True)
            nc.scalar.activation(hidT[:], p_hid[0:64, 0:BLOC],
                                 mybir.ActivationFunctionType.Prelu, alpha=0.1)
            p_k1 = pmlp.tile([64, 512], F32, tag="pmlp")
            p_k2 = pmlp.tile([64, 512], F32, tag="pmlp")
            nc.tensor.matmul(p_k1[0:BLOC, 0:288], hidT[:], w2t_t[:, 0:288],
                             start=True, stop=True)
            nc.tensor.matmul(p_k2[0:BLOC, 0:288], hidT[:], w2t_t[:, 288:576],
                             start=True, stop=True)
            nc.scalar.copy(kern_flat[:, 0:288], p_k1[0:BLOC, 0:288])
            nc.scalar.copy(kern_flat[:, 288:576], p_k2[0:BLOC, 0:288])
            # reshape (BLOC, 576) -> (128, 9): partition bc = b*64+c
            for b in range(BLOC):
                nc.gpsimd.dma_start(kern128[b * 64:(b + 1) * 64, :],
                                    kern_flat[b:b + 1, :])
            # fill diagonal blocks of wbuf: cols t*128+64 .. t*128+128
            for t in range(9):
                nc.vector.tensor_scalar(
                    wbuf[:, t * 128 + 64:t * 128 + 128], id_t[:],
                    kern128[:, t:t + 1], None, op0=mybir.AluOpType.mult)

            # ---- main loop over 96 spatial tiles (2 padded rows each) ----
            for ti in range(NT):
                hp0 = 1 + 2 * ti           # first padded row of this tile
                qs = hp0 * WP              # padded flat index of tile start
                pb0 = pmain.tile([128, NTILE], F32, tag="pb0")
                pb1 = pmain.tile([128, NTILE], F32, tag="pb1")
                for t, (dy, dx) in enumerate(TAPS):
                    base = G + qs + dy * WP + dx
                    nc.tensor.matmul(
                        pb0[:], wbuf[0:64, t * 128:(t + 1) * 128],
                        x_pad[0:64, base:base + NTILE],
                        start=(t == 0), stop=False, tile_position=(0, 0))
                    nc.tensor.matmul(
                        pb1[:], wbuf[64:128, t * 128:(t + 1) * 128],
                        x_pad[64:128, base:base + NTILE],
                        start=(t == 0), stop=False, tile_position=(64, 0))
                dst0 = work.tile([128, NTILE], F32R, tag="dst0")
                dst1 = work.tile([128, NTILE], F32R, tag="dst1")
                nc.scalar.activation(dst0[64:128, :], pb0[64:128, :],
                                     mybir.ActivationFunctionType.Prelu,
                                     alpha=0.1)
                nc.scalar.activation(dst1[64:128, :], pb1[64:128, :],
                                     mybir.ActivationFunctionType.Prelu,
                                     alpha=0.1)
                nc.tensor.matmul(pb0[0:64, :], id_t[64:128, :], dst0[64:128, :],
                                 start=False, stop=True, tile_position=(64, 0))
                nc.tensor.matmul(pb1[0:64, :], id_t[64:128, :], dst1[64:128, :],
                                 start=False, stop=True, tile_position=(64, 0))
                # final eviction: compact away pad columns, add bias
                st0 = work.tile([64, 2, W], F32, tag="st0")
                st1 = work.tile([64, 2, W], F32, tag="st1")
                pv0 = pb0[0:64, :].rearrange("p (r w) -> p r w", w=WP)[:, :, 1:1 + W]
                pv1 = pb1[0:64, :].rearrange("p (r w) -> p r w", w=WP)[:, :, 1:1 + W]
                nc.vector.tensor_scalar(st0[:], pv0, bias_t[0:64, :], None,
                                        op0=mybir.AluOpType.add)
                nc.vector.tensor_scalar(st1[:], pv1, bias_t[0:64, :], None,
                                        op0=mybir.AluOpType.add)
                h0 = 2 * ti
                nc.sync.dma_start(
                    out[0:64, h0 * W:(h0 + 2) * W].rearrange(
                        "p (r w) -> p r w", w=W), st0[:])
                nc.sync.dma_start(
                    out[64:128, h0 * W:(h0 + 2) * W].rearrange(
                        "p (r w) -> p r w", w=W), st1[:])

    _legalize_waits(nc)
    return nc


_NC_CACHE = None


def _get_nc():
    global _NC_CACHE
    if _NC_CACHE is None:
        _NC_CACHE = _build_nc()
    return _NC_CACHE


def kernel(x, k_v, W1, W2, conv_w, conv_b):
    from concourse.bass_utils import run_bass_kernel_spmd

    x = np.ascontiguousarray(x, dtype=np.float32)
    k_v = np.ascontiguousarray(k_v, dtype=np.float32)
    W1 = np.ascontiguousarray(W1, dtype=np.float32)
    W2 = np.ascontiguousarray(W2, dtype=np.float32)
    conv_w = np.ascontiguousarray(conv_w, dtype=np.float32)
    conv_b = np.ascontiguousarray(conv_b, dtype=np.float32)

    # host-side weight layout prep (parameters only; no input-dependent math)
    blocks = []
    for dy in (0, 1, 2):
        for dx in (0, 1, 2):
            lhsT = conv_w[:, :, dy, dx].T          # [ci, co]
            blocks.append(np.concatenate(
                [lhsT, np.zeros((64, 64), np.float32)], axis=1))
    wstat_half = np.concatenate(blocks, axis=1)     # [64, 9*128]
    wstat = np.tile(wstat_half, (2, 1))             # [128, 9*128]
    ident = np.tile(np.eye(64, dtype=np.float32), (2, 1))
    biasd = np.tile(conv_b, 2)[:, None].astype(np.float32)
    w1t = W1.T.copy()                               # [64, 64]
    w2t = W2.T.copy()                               # [64, 576]

    in_maps = []
    for c in range(NCORES):
        xs = x[c * BLOC:(c + 1) * BLOC].reshape(128, HW)
        kvT = k_v[c * BLOC:(c + 1) * BLOC].T.copy()  # [64, BLOC]
        in_maps.append({
            "xs": xs, "kvT": kvT, "w1t": w1t, "w2t": w2t,
            "wstat": wstat, "ident": ident, "biasd": biasd,
        })

    nc = _get_nc()
    res = run_bass_kernel_spmd(nc, in_maps, core_ids=list(range(NCORES)))
    out = np.empty((B, C, H, W), dtype=np.float32)
    for c in range(NCORES):
        out[c * BLOC:(c + 1) * BLOC] = res.results[c]["out"].reshape(
            BLOC, C, H, W)
    return out
